# revision 24
# baseline (speedup 1.0000x reference)
"""BandSplit (BSRNN) Trainium2 kernel, fp16 edition.

Math per band k (31 bands over 257 freq bins, band widths 3/6/16/27):
  xg = x[b, :, band_bins, t] flattened to d = 2*bw features (torch order:
       bin-major, re/im minor)
  out[b, k, t, :] = LayerNorm_d(xg) @ W_k + b_k          (d -> C=128)

Algebraic refactor (per band, per t), with q = x * rstd:
  out = q @ (Wg - colmean_d(Wg)) + bb
  with host-precomputed  Wg = gamma*W,  bb = b + beta @ W.

fp16 strategy: the kernel is DMA-bound (output is 31*3000*128 floats per
core), so x / params / staging / output all move as fp16 (quantization
error ~1e-3 rel, inside the 2e-2 gate and below the old fp32r truncation
error).  The output DRAM layout is [T, 31, C] so each DMA descriptor is
one t-row of 31*128 contiguous fp16 = 7936 B (>=512 B keeps the DMA bus
at full rate); the host transposes back to [31, T, C] fp32.  ALL matmuls
are fp16 (mixing fp32r and fp16 matmuls corrupts the PE datapath on real
HW).  fp16 matmuls run 1 cycle/row at any width and may write PSUM at
32-aligned bases, so per-band stats land directly in quadrant layout.

Packs are sized so each pack's (n*C)-wide output splits into equal-width
PSUM-bank blocks; each pack (or pack pair) drains with a single Act/DVE
instruction over a 2-level access pattern, cutting the per-instruction
SBUF/PSUM access overhead that otherwise dominates the vector engines.

Sharding: batch-parallel, core b handles x[b] (B=8 = n_cores).
"""

import numpy as np

T = 3000
C = 128
F_BINS = 257
EPS = 1e-5
GROUPS = [(10, 3), (12, 6), (8, 16), (1, 27)]  # (n_bands, bins_per_band)

SPAN = 512   # stats/prep span (free dim of PSUM bank) == x-slab width
CHUNK = 128  # output t-chunk (PSUM partition dim)


# ---------------------------------------------------------------- metadata --
class Band:
    def __init__(self, g, i, f0, bw):
        self.g, self.i, self.f0, self.bw = g, i, f0, bw


BANDS = []
_f0 = 0
for _g, (_n, _bw) in enumerate(GROUPS):
    for _i in range(_n):
        BANDS.append(Band(_g, _i, _f0, _bw))
        _f0 += _bw
assert _f0 == F_BINS and len(BANDS) == 31


class Pack:
    def __init__(self, pid, band_ids, qset, quad, blocks):
        self.pid = pid
        self.band_ids = list(band_ids)
        self.n = len(self.band_ids)
        self.bws = [BANDS[k].bw for k in self.band_ids]
        self.offs = list(np.cumsum([0] + self.bws[:-1]))  # bin offset in pack
        self.F2 = sum(self.bws)                           # bins in pack
        self.F = 2 * self.F2                              # feature rows
        self.K = self.F + 1                               # + ones row
        self.K32 = (self.K + 31) // 32 * 32               # tile partitions
        self.k0 = self.band_ids[0]                        # first global band
        self.f0 = BANDS[self.k0].f0                       # first freq bin
        self.qset = qset                                  # 'A' or 'B'
        self.quad = quad                                  # PSUM row base /32
        self.blocks = blocks      # [(bank, col_off_in_pack, width)]
        assert self.K <= 128 and self.n <= 32
        assert sum(w for _, _, w in blocks) == self.n * C


# matmul tile_position bases are limited to {0, 32, 64} (quadrant-3 HW bug),
# so at most 3 packs share a stats/srstd tile set.  Pack sizes are chosen so
# every pack's output is equal-width blocks (enables single-instr drains).
PACKS = [
    Pack(0, range(0, 9), 'A', 0,   # 9x3-bin: 1152 cols = 3x384
         [(0, 0, 384), (1, 384, 384), (2, 768, 384)]),
    Pack(1, range(9, 16), 'A', 1,  # 1x3 + 6x6-bin: 896 = 2x448
         [(0, 0, 448), (1, 448, 448)]),
    Pack(2, range(16, 22), 'A', 2,  # 6x6-bin: 768 = 2x384
         [(0, 0, 384), (1, 384, 384)]),
    Pack(3, range(22, 25), 'B', 0,  # 3x16-bin: 384
         [(0, 0, 384)]),
    Pack(4, range(25, 28), 'B', 1,  # 3x16-bin: 384 (bank 1 of shared tile)
         [(1, 0, 384)]),
    Pack(5, range(28, 31), 'B', 2,  # 2x16 + 27-bin: 384
         [(0, 0, 384)]),
]
QSETS = "AB"
EREP_COLS = max(p.F for p in PACKS)  # 118

# drain groups: packs sharing one multi-bank PSUM tile, drained by ONE
# Act/DVE instruction via a 2-level (bank, width) access pattern.
# (pack_ids, nbanks, width_per_bank, engine)
DRAIN_GROUPS = [
    ((0,), 3, 384, "scalar"),
    ((1,), 2, 448, "scalar"),
    ((2,), 2, 384, "vector"),
    ((3, 4), 2, 384, "vector"),
    ((5,), 1, 384, "scalar"),
]

import os as _os

# schedule knobs (env-overridable for offline tuning; defaults are tuned)
WARMUP = int(_os.environ.get("K_WARMUP", "8"))
DUMMY_SQRT = int(_os.environ.get("K_DUMMY_SQRT", "1"))
SLAB0W = _os.environ.get("K_SLAB0GROUPS", "512,512")
SLAB3D = int(_os.environ.get("K_SLAB3D", "1"))

SPANS = [(s0, min(SPAN, T - s0)) for s0 in range(0, T, SPAN)]
OB_COLS = 31 * C  # 3968


def _round_f32r(a):
    """RNE-round fp32 to fp32r (11-bit mantissa; low 12 bits zero)."""
    a = np.ascontiguousarray(np.asarray(a, np.float32))
    u = a.view(np.uint32).copy()
    lsb = (u >> 12) & 1
    u = u + 0x7FF + lsb
    u &= np.uint32(0xFFFFF000)
    return u.view(np.float32)


# ------------------------------------------------------------- host params --
def _host_params(inputs):
    f32 = np.float32
    wextall = np.zeros((128, OB_COLS), f32)
    selall = np.zeros((128, 32 * len(PACKS)), f32)
    erepall = np.zeros((96, 2 * EREP_COLS), f32)
    for p in PACKS:
        qi = QSETS.index(p.qset)
        for j, k in enumerate(p.band_ids):
            b = BANDS[k]
            bw, d = b.bw, 2 * b.bw
            W = np.asarray(inputs[f"g{b.g}_W"][b.i], f32)        # (d, C)
            gam = np.asarray(inputs[f"g{b.g}_gamma"][b.i], f32)  # (d,)
            bet = np.asarray(inputs[f"g{b.g}_beta"][b.i], f32)
            bias = np.asarray(inputs[f"g{b.g}_b"][b.i], f32)     # (C,)
            Wg = gam[:, None] * W
            Wg = Wg - Wg.mean(0, keepdims=True)  # fold -mean(q)*u into Wg
            cols = slice(k * C, (k + 1) * C)
            # device row layout is plane-major: row pl*F2 + off_j + f
            # holds (plane pl, bin f) of band j == torch feature 2f+pl
            for pl in range(2):
                r0 = pl * p.F2 + p.offs[j]
                wextall[r0:r0 + bw, cols] = Wg[2 * np.arange(bw) + pl]
                selall[r0:r0 + bw, 32 * p.pid + j] = 1.0 / d
                erepall[32 * p.quad + j,
                        qi * EREP_COLS + r0:qi * EREP_COLS + r0 + bw] = 1.0
            wextall[p.F, cols] = bias + bet @ W
    return {"wextall": wextall.astype(np.float16),
            "selall": selall.astype(np.float16),
            "erepall": erepall.astype(np.float16)}


# ------------------------------------------------------------ device build --
_CACHE = {}


def _build():
    if "nc" in _CACHE:
        return _CACHE["nc"]
    import concourse.bacc as bacc
    import concourse.tile as tile
    from concourse import mybir

    Alu = mybir.AluOpType
    Act = mybir.ActivationFunctionType
    F32 = mybir.dt.float32
    F16 = mybir.dt.float16

    nc = bacc.Bacc("TRN2", target_bir_lowering=False, debug=False, num_devices=8)
    x_d = nc.dram_tensor("x", [2, F_BINS, T], F16, kind="ExternalInput")
    out_d = nc.dram_tensor("out", [T, 31, C], F16, kind="ExternalOutput")
    wext_d = nc.dram_tensor("wextall", [128, OB_COLS], F16, kind="ExternalInput")
    selx_d = nc.dram_tensor("selall", [128, 32 * len(PACKS)], F16,
                            kind="ExternalInput")
    erep_d = nc.dram_tensor("erepall", [96, 2 * EREP_COLS], F16,
                            kind="ExternalInput")

    with tile.TileContext(nc) as tc:
        with (
            tc.tile_pool(name="const", bufs=1) as const,
            tc.tile_pool(name="xsqp", bufs=7) as xsqp,
            tc.tile_pool(name="stt", bufs=2) as stt,
            tc.tile_pool(name="obp",
                         bufs=int(_os.environ.get("K_OBP", "3"))) as obp,
            tc.tile_pool(name="srp", bufs=2) as srp,
            tc.tile_pool(name="stps", bufs=1, space="PSUM") as stps,
            tc.tile_pool(name="outps",
                         bufs=int(_os.environ.get("K_OPB", "2")),
                         space="PSUM") as outps,
        ):
            # ---- resident constants
            selall = const.tile([128, 32 * len(PACKS)], F16, tag="sel", name="selall")
            nc.sync.dma_start(out=selall[:], in_=selx_d[:])
            erepall = const.tile([96, 2 * EREP_COLS], F16, tag="er", name="erepall")
            nc.sync.dma_start(out=erepall[:], in_=erep_d[:])
            xin = {}
            for p in PACKS:
                xin[p.pid] = const.tile([p.F, T], F16, tag=f"xin{p.pid}",
                                        name=f"xin{p.pid}")

            def load_slab(s0, sw):
                for p in PACKS:
                    if SLAB3D:
                        # one DMA per pack covering both planes: src is a 3D
                        # [2, F2, sw] DRAM AP, dst a plane-major [F, sw] AP
                        s_ = x_d[:, p.f0:p.f0 + p.F2, s0:s0 + sw]
                        d_ = xin[p.pid][0:p.F, s0:s0 + sw]
                        nc.sync.dma_start(out=d_, in_=s_)
                    else:
                        for pl in range(2):
                            s_ = x_d[pl, p.f0:p.f0 + p.F2, s0:s0 + sw]
                            d_ = xin[p.pid][pl * p.F2:(pl + 1) * p.F2,
                                            s0:s0 + sw]
                            nc.sync.dma_start(out=d_, in_=s_)

            # startup: first group covers span 0 only so stats can start
            # ASAP; wext next (span-0 mains need it); then the span-1 slab.
            _groups = [int(g) for g in SLAB0W.split(",") if g]
            assert sum(_groups) <= T
            wext = const.tile([128, OB_COLS], F16, tag="wx", name="wextall")
            _g0 = 0
            for _gi, _gw in enumerate(_groups):
                load_slab(_g0, _gw)
                _g0 += _gw
                if _gi == 0:
                    nc.sync.dma_start(out=wext[:], in_=wext_d[:])
            _s0w = 0  # spans fully covered by startup groups
            _acc = 0
            for _s0, _sw in SPANS:
                if _acc + _sw <= _g0:
                    _acc += _sw
                    _s0w += 1
                else:
                    break

            def selx(p):
                return selall[0:p.F, 32 * p.pid:32 * p.pid + 32]

            def erep(p):
                qi = QSETS.index(p.qset)
                q0 = 32 * p.quad
                return erepall[q0:q0 + p.n,
                               qi * EREP_COLS:qi * EREP_COLS + p.F]

            eps_t = const.tile([128, 1], F32, tag="epsc", name="epsc")
            nc.vector.memset(eps_t[:], EPS)
            ones_t = const.tile([32, SPAN], F16, tag="ones", name="ones")
            nc.vector.memset(ones_t[:], 1.0)
            # PE p-state warm-up: keep the PE busy during the initial x-slab
            # load so span-0 matmuls run at full clock.
            for _ in range(WARMUP):
                wm = outps.tile([128, 1536], F32, tag="op", name="wm")
                nc.tensor.matmul(
                    wm[0:128, 0:192], ones_t[0:32, 0:128],
                    ones_t[0:32, 0:192], start=True, stop=True)
            if DUMMY_SQRT:
                # dummy Sqrt: makes the compiler pick the sqrt-bearing
                # activation table up front (it also holds Square/Copy)
                scr0 = const.tile([1, 1], F32, tag="scr0", name="scr0")
                nc.scalar.activation(scr0[0:1, 0:1], eps_t[0:1, 0:1],
                                     Act.Sqrt, bias=eps_t[0:1, 0:1],
                                     scale=1.0)

            # persistent double-buffered lhsT tiles; the ones row (row F)
            # is written once via 32-aligned copies (clobbered x rows are
            # rewritten by the per-span q prep before use).
            xpt = {}
            for i, p in enumerate(PACKS):
                for par in range(2):
                    t_ = const.tile([p.K32, SPAN], F16, tag=f"xp{p.pid}_{par}",
                                    name=f"xp{p.pid}_{par}")
                    xpt[(p.pid, par)] = t_
                    for m0 in range(p.F // 32 * 32, p.K32, 32):
                        if (i + par) % 2 == 0:
                            nc.vector.tensor_copy(t_[m0:m0 + 32, :],
                                                  ones_t[0:32, :])
                        else:
                            nc.scalar.activation(t_[m0:m0 + 32, :],
                                                 ones_t[0:32, :], Act.Copy)

            from concourse.dve_ops import (
                RECIP_APPROX_FAST_CONSTS as _RC,
                RECIPROCAL_APPROX_FAST as _RF,
            )
            srstd_by_si = {}

            def emit_stats(si, qs):
                """Stats + rstd chain for span si, ONE qset (3 packs).  The
                two qsets serialize through single-buffered [96, SPAN] PSUM
                tiles (2 banks total), and are emitted in different chunk
                slots so the PE never stalls on the Act/DVE tail."""
                s0, sw = SPANS[si]
                srstd16 = srp.tile([96, SPAN], F16, tag=f"sr16{qs}",
                                   name=f"sr16{qs}")
                srstd_by_si.setdefault(si, {})[qs] = srstd16
                mu_ps = stps.tile([96, SPAN], F32, tag="mu", name="mu")
                msq_ps = stps.tile([96, SPAN], F32, tag="ms", name="ms")
                for p in PACKS:
                    if p.qset != qs:
                        continue
                    q0 = 32 * p.quad
                    xin_f = xin[p.pid][:, s0:s0 + sw]
                    # x^2 on GPSIMD (SBUF->SBUF is legal there), freeing the
                    # scalar engine for PSUM drains
                    xsq = xsqp.tile([128, SPAN], F16, tag="xsq", name="xsq")
                    nc.gpsimd.tensor_tensor(
                        xsq[0:p.F, :sw], xin_f, xin_f, op=Alu.mult)
                    nc.tensor.matmul(
                        mu_ps[q0:q0 + 32, :sw], selx(p),
                        xin_f, start=True, stop=True)
                    nc.tensor.matmul(
                        msq_ps[q0:q0 + 32, :sw], selx(p),
                        xsq[0:p.F, :sw], start=True, stop=True)
                # rstd = rsqrt(msq - mu^2 + eps), batched over the qset
                musq = stt.tile([96, SPAN], F32, tag=f"musq{qs}",
                                name=f"musq{qs}")
                nc.scalar.activation(
                    musq[0:96, :sw], mu_ps[0:96, :sw], Act.Square)
                var = stt.tile([96, SPAN], F32, tag="var", name="var")
                nc.vector.tensor_tensor(
                    var[0:96, :sw], msq_ps[0:96, :sw],
                    musq[0:96, :sw], op=Alu.subtract)
                sq = stt.tile([96, SPAN], F32, tag="sq", name="sq")
                nc.scalar.activation(
                    sq[0:96, :sw], var[0:96, :sw], Act.Sqrt,
                    bias=eps_t[0:96, 0:1], scale=1.0)
                srstd = stt.tile([96, SPAN], F32, tag="sr", name="sr")
                nc.vector._custom_dve(
                    _RF, out=srstd[0:96, :sw],
                    in0=sq[0:96, :sw], s0=_RC["s0"], s1=_RC["s1"],
                    imm2=_RC["imm2"])
                # fp16 copy for the (all-fp16) erep broadcast matmul;
                # on GPSIMD to keep Act/DVE free for drains
                nc.gpsimd.tensor_copy(srstd16[0:96, :sw],
                                      srstd[0:96, :sw])

            def emit_prep(si):
                """Pack lhsT prep for span si: q = x * rstd_rep, into the
                parity-si xpt tiles.  Emitted during span si-1 (the tiles
                are double-buffered) so span si's mains start immediately."""
                s0, sw = SPANS[si]
                par = si % 2
                srstd16 = srstd_by_si.pop(si)
                for p in PACKS:
                    q0 = 32 * p.quad
                    t_ = xpt[(p.pid, par)]
                    rr = outps.tile([128, 1536], F32, tag="op", name="rr")
                    nc.tensor.matmul(
                        rr[0:p.F, 0:sw],
                        erep(p),
                        srstd16[p.qset][q0:q0 + p.n, :sw],
                        start=True, stop=True)
                    nc.vector.tensor_tensor(
                        t_[0:p.F, :sw], xin[p.pid][:, s0:s0 + sw],
                        rr[0:p.F, 0:sw], op=Alu.mult)

            emit_stats(0, 'A')
            emit_stats(0, 'B')
            emit_prep(0)
            for si, (s0, sw) in enumerate(SPANS):
                par = si % 2
                # ---- main matmuls + merged drains + one DMA per t-chunk;
                # next span's slab/stats interleave between chunks so the
                # in-order engine queues never batch them behind a whole
                # span of drains.
                for ci, c0 in enumerate(range(s0, s0 + sw, CHUNK)):
                    cw = min(CHUNK, s0 + sw - c0)
                    ob = obp.tile([128, OB_COLS], F16, tag="ob", name="ob")
                    for pids, nb, w, eng in DRAIN_GROUPS:
                        op = outps.tile([128, 1536], F32, tag="op", name="op")
                        for pid in pids:
                            p = PACKS[pid]
                            lhsT = xpt[(p.pid, par)][0:p.K,
                                                     c0 - s0:c0 - s0 + cw]
                            for (bank, poff, bw_) in p.blocks:
                                wcol = p.k0 * C + poff
                                nc.tensor.matmul(
                                    op[0:cw, 512 * bank:512 * bank + bw_],
                                    lhsT,
                                    wext[0:p.K, wcol:wcol + bw_],
                                    start=True, stop=True)
                        gcol = PACKS[pids[0]].k0 * C
                        src = op[0:cw, 0:512 * nb].rearrange(
                            "t (b x) -> t b x", b=nb)[:, :, 0:w]
                        dst = ob[0:cw, gcol:gcol + nb * w].rearrange(
                            "t (b x) -> t b x", b=nb)
                        if eng == "scalar":
                            nc.scalar.activation(dst, src, Act.Copy)
                        else:
                            nc.vector.tensor_copy(dst, src)
                    nc.sync.dma_start(
                        out=out_d[c0:c0 + cw, :, :].rearrange(
                            "t j c -> t (j c)"),
                        in_=ob[0:cw, :])
                    if ci == 0 and _s0w <= si + 2 < len(SPANS):
                        load_slab(*SPANS[si + 2])
                    if ci == 1 and si + 1 < len(SPANS):
                        emit_stats(si + 1, 'A')
                    if ci == 2 and si + 1 < len(SPANS):
                        emit_stats(si + 1, 'B')
                if si + 1 < len(SPANS):
                    emit_prep(si + 1)

    nc.compile()
    _CACHE["nc"] = nc
    return nc


# ------------------------------------------------------------------ driver --
def kernel(**inputs):
    from concourse.bass_utils import run_bass_kernel_spmd

    x = np.asarray(inputs["x"]).astype(np.float16)
    B = x.shape[0]
    assert x.shape == (8, 2, F_BINS, T)
    ext = _host_params(inputs)
    nc = _build()
    in_maps = []
    for b in range(B):
        m = {"x": x[b]}
        m.update(ext)
        in_maps.append(m)
    res = run_bass_kernel_spmd(nc, in_maps, core_ids=list(range(8)))
    out = np.stack([res.results[b]["out"].transpose(1, 0, 2)
                    for b in range(B)], axis=0)
    return out.astype(np.float32)


# revision 28
# speedup vs baseline: 1.3213x; 1.3213x over previous
"""BandSplit (BSRNN) Trainium2 kernel, fp16 edition.

Math per band k (31 bands over 257 freq bins, band widths 3/6/16/27):
  xg = x[b, :, band_bins, t] flattened to d = 2*bw features (torch order:
       bin-major, re/im minor)
  out[b, k, t, :] = LayerNorm_d(xg) @ W_k + b_k          (d -> C=128)

Algebraic refactor (per band, per t), with q = x * rstd:
  out = q @ (Wg - colmean_d(Wg)) + bb
  with host-precomputed  Wg = gamma*W,  bb = b + beta @ W.

fp16 strategy: the kernel is DMA-bound (output is 31*3000*128 floats per
core), so x / params / staging / output all move as fp16 (quantization
error ~1e-3 rel, inside the 2e-2 gate and below the old fp32r truncation
error).  The output DRAM layout is [T, 31, C] so each DMA descriptor is
one t-row of 31*128 contiguous fp16 = 7936 B (>=512 B keeps the DMA bus
at full rate); the host transposes back to [31, T, C] fp32.  ALL matmuls
are fp16 (mixing fp32r and fp16 matmuls corrupts the PE datapath on real
HW).  fp16 matmuls run 1 cycle/row at any width and may write PSUM at
32-aligned bases, so per-band stats land directly in quadrant layout.

Packs are sized so each pack's (n*C)-wide output splits into equal-width
PSUM-bank blocks; each pack (or pack pair) drains with a single Act/DVE
instruction over a 2-level access pattern, cutting the per-instruction
SBUF/PSUM access overhead that otherwise dominates the vector engines.

Sharding: batch-parallel, core b handles x[b] (B=8 = n_cores).
"""

import numpy as np

T = 3000
C = 128
F_BINS = 257
EPS = 1e-5
GROUPS = [(10, 3), (12, 6), (8, 16), (1, 27)]  # (n_bands, bins_per_band)

SPAN = 512   # stats/prep span (free dim of PSUM bank) == x-slab width
CHUNK = 128  # output t-chunk (PSUM partition dim)


# ---------------------------------------------------------------- metadata --
class Band:
    def __init__(self, g, i, f0, bw):
        self.g, self.i, self.f0, self.bw = g, i, f0, bw


BANDS = []
_f0 = 0
for _g, (_n, _bw) in enumerate(GROUPS):
    for _i in range(_n):
        BANDS.append(Band(_g, _i, _f0, _bw))
        _f0 += _bw
assert _f0 == F_BINS and len(BANDS) == 31


class Pack:
    def __init__(self, pid, band_ids, qset, quad, blocks):
        self.pid = pid
        self.band_ids = list(band_ids)
        self.n = len(self.band_ids)
        self.bws = [BANDS[k].bw for k in self.band_ids]
        self.offs = list(np.cumsum([0] + self.bws[:-1]))  # bin offset in pack
        self.F2 = sum(self.bws)                           # bins in pack
        self.F = 2 * self.F2                              # feature rows
        self.K = self.F + 1                               # + ones row
        self.K32 = (self.K + 31) // 32 * 32               # tile partitions
        self.k0 = self.band_ids[0]                        # first global band
        self.f0 = BANDS[self.k0].f0                       # first freq bin
        self.qset = qset                                  # 'A' or 'B'
        self.quad = quad                                  # PSUM row base /32
        self.blocks = blocks      # [(bank, col_off_in_pack, width)]
        assert self.K <= 128 and self.n <= 32
        assert sum(w for _, _, w in blocks) == self.n * C


# matmul tile_position bases are limited to {0, 32, 64} (quadrant-3 HW bug),
# so at most 3 packs share a stats/srstd tile set.  Pack sizes are chosen so
# every pack's output is equal-width blocks (enables single-instr drains).
PACKS = [
    Pack(0, range(0, 9), 'A', 0,   # 9x3-bin: 1152 cols = 3x384
         [(0, 0, 384), (1, 384, 384), (2, 768, 384)]),
    Pack(1, range(9, 16), 'A', 1,  # 1x3 + 6x6-bin: 896 = 2x448
         [(0, 0, 448), (1, 448, 448)]),
    Pack(2, range(16, 22), 'A', 2,  # 6x6-bin: 768 = 2x384
         [(0, 0, 384), (1, 384, 384)]),
    Pack(3, range(22, 25), 'B', 0,  # 3x16-bin: 384
         [(0, 0, 384)]),
    Pack(4, range(25, 28), 'B', 1,  # 3x16-bin: 384 (bank 1 of shared tile)
         [(1, 0, 384)]),
    Pack(5, range(28, 31), 'B', 2,  # 2x16 + 27-bin: 384
         [(0, 0, 384)]),
]
QSETS = "AB"
EREP_COLS = max(p.F for p in PACKS)  # 118

# drain groups: packs sharing one multi-bank PSUM tile, drained by ONE
# Act/DVE instruction via a 2-level (bank, width) access pattern.
# (tile_kind, [(pid, [(bank, pack_col_off, width)])], nbanks, width, ob_col,
#  engine) -- engines balanced incl. the fixed stats/xmult loads.
DRAIN_GROUPS = [
    ("op2", [(0, [(0, 0, 384), (1, 384, 384)])], 2, 384, 0, "vector"),
    ("op1", [(0, [(0, 768, 384)])], 1, 384, 768, "scalar"),
    ("op2", [(1, [(0, 0, 448), (1, 448, 448)])], 2, 448, 1152, "scalar"),
    ("op2", [(2, [(0, 0, 384), (1, 384, 384)])], 2, 384, 2048, "scalar"),
    ("op2", [(3, [(0, 0, 384)]), (4, [(1, 0, 384)])], 2, 384, 2816, "vector"),
    ("op1", [(5, [(0, 0, 384)])], 1, 384, 3584, "scalar"),
]

import os as _os

# schedule knobs (env-overridable for offline tuning; defaults are tuned)
WARMUP = int(_os.environ.get("K_WARMUP", "8"))
DUMMY_SQRT = int(_os.environ.get("K_DUMMY_SQRT", "1"))
SLAB0W = _os.environ.get("K_SLAB0GROUPS", "512,512")
SLAB3D = int(_os.environ.get("K_SLAB3D", "1"))

SPANS = [(s0, min(SPAN, T - s0)) for s0 in range(0, T, SPAN)]
OB_COLS = 31 * C  # 3968


def _round_f32r(a):
    """RNE-round fp32 to fp32r (11-bit mantissa; low 12 bits zero)."""
    a = np.ascontiguousarray(np.asarray(a, np.float32))
    u = a.view(np.uint32).copy()
    lsb = (u >> 12) & 1
    u = u + 0x7FF + lsb
    u &= np.uint32(0xFFFFF000)
    return u.view(np.float32)


# ------------------------------------------------------------- host params --
def _host_params(inputs):
    f32 = np.float32
    wextall = np.zeros((128, OB_COLS), f32)
    selall = np.zeros((128, 32 * len(PACKS)), f32)
    erepall = np.zeros((96, 2 * EREP_COLS), f32)
    for p in PACKS:
        qi = QSETS.index(p.qset)
        for j, k in enumerate(p.band_ids):
            b = BANDS[k]
            bw, d = b.bw, 2 * b.bw
            W = np.asarray(inputs[f"g{b.g}_W"][b.i], f32)        # (d, C)
            gam = np.asarray(inputs[f"g{b.g}_gamma"][b.i], f32)  # (d,)
            bet = np.asarray(inputs[f"g{b.g}_beta"][b.i], f32)
            bias = np.asarray(inputs[f"g{b.g}_b"][b.i], f32)     # (C,)
            Wg = gam[:, None] * W
            Wg = Wg - Wg.mean(0, keepdims=True)  # fold -mean(q)*u into Wg
            cols = slice(k * C, (k + 1) * C)
            # device row layout is plane-major: row pl*F2 + off_j + f
            # holds (plane pl, bin f) of band j == torch feature 2f+pl
            for pl in range(2):
                r0 = pl * p.F2 + p.offs[j]
                wextall[r0:r0 + bw, cols] = Wg[2 * np.arange(bw) + pl]
                selall[r0:r0 + bw, 32 * p.pid + j] = 1.0 / d
                erepall[32 * p.quad + j,
                        qi * EREP_COLS + r0:qi * EREP_COLS + r0 + bw] = 1.0
            wextall[p.F, cols] = bias + bet @ W
    return {"wextall": wextall.astype(np.float16),
            "selall": selall.astype(np.float16),
            "erepall": erepall.astype(np.float16)}


# ------------------------------------------------------------ device build --
_CACHE = {}


def _build():
    if "nc" in _CACHE:
        return _CACHE["nc"]
    import concourse.bacc as bacc
    import concourse.tile as tile
    from concourse import mybir

    Alu = mybir.AluOpType
    Act = mybir.ActivationFunctionType
    F32 = mybir.dt.float32
    F16 = mybir.dt.float16

    nc = bacc.Bacc("TRN2", target_bir_lowering=False, debug=False, num_devices=8)
    x_d = nc.dram_tensor("x", [2, F_BINS, T], F16, kind="ExternalInput")
    out_d = nc.dram_tensor("out", [T, 31, C], F16, kind="ExternalOutput")
    wext_d = nc.dram_tensor("wextall", [128, OB_COLS], F16, kind="ExternalInput")
    selx_d = nc.dram_tensor("selall", [128, 32 * len(PACKS)], F16,
                            kind="ExternalInput")
    erep_d = nc.dram_tensor("erepall", [96, 2 * EREP_COLS], F16,
                            kind="ExternalInput")

    with tile.TileContext(nc) as tc:
        with (
            tc.tile_pool(name="const", bufs=1) as const,
            tc.tile_pool(name="xsqp", bufs=7) as xsqp,
            tc.tile_pool(name="stt", bufs=2) as stt,
            tc.tile_pool(name="obp",
                         bufs=int(_os.environ.get("K_OBP", "3"))) as obp,
            tc.tile_pool(name="srp", bufs=2) as srp,
            tc.tile_pool(name="stps", bufs=1, space="PSUM") as stps,
            tc.tile_pool(name="ops2", bufs=2, space="PSUM") as ops2,
            tc.tile_pool(name="ops1", bufs=2, space="PSUM") as ops1,
        ):
            # ---- resident constants
            selall = const.tile([128, 32 * len(PACKS)], F16, tag="sel", name="selall")
            nc.sync.dma_start(out=selall[:], in_=selx_d[:])
            erepall = const.tile([96, 2 * EREP_COLS], F16, tag="er", name="erepall")
            nc.sync.dma_start(out=erepall[:], in_=erep_d[:])
            xin = {}
            for p in PACKS:
                xin[p.pid] = const.tile([p.F, T], F16, tag=f"xin{p.pid}",
                                        name=f"xin{p.pid}")

            def load_slab(s0, sw):
                for p in PACKS:
                    if SLAB3D:
                        # one DMA per pack covering both planes: src is a 3D
                        # [2, F2, sw] DRAM AP, dst a plane-major [F, sw] AP
                        s_ = x_d[:, p.f0:p.f0 + p.F2, s0:s0 + sw]
                        d_ = xin[p.pid][0:p.F, s0:s0 + sw]
                        nc.sync.dma_start(out=d_, in_=s_)
                    else:
                        for pl in range(2):
                            s_ = x_d[pl, p.f0:p.f0 + p.F2, s0:s0 + sw]
                            d_ = xin[p.pid][pl * p.F2:(pl + 1) * p.F2,
                                            s0:s0 + sw]
                            nc.sync.dma_start(out=d_, in_=s_)

            # startup: first group covers span 0 only so stats can start
            # ASAP; wext next (span-0 mains need it); then the span-1 slab.
            _groups = [int(g) for g in SLAB0W.split(",") if g]
            assert sum(_groups) <= T
            wext = const.tile([128, OB_COLS], F16, tag="wx", name="wextall")
            _g0 = 0
            for _gi, _gw in enumerate(_groups):
                load_slab(_g0, _gw)
                _g0 += _gw
                if _gi == 0:
                    nc.sync.dma_start(out=wext[:], in_=wext_d[:])
            _s0w = 0  # spans fully covered by startup groups
            _acc = 0
            for _s0, _sw in SPANS:
                if _acc + _sw <= _g0:
                    _acc += _sw
                    _s0w += 1
                else:
                    break

            def selx(p):
                return selall[0:p.F, 32 * p.pid:32 * p.pid + 32]

            def erep(p):
                qi = QSETS.index(p.qset)
                q0 = 32 * p.quad
                return erepall[q0:q0 + p.n,
                               qi * EREP_COLS:qi * EREP_COLS + p.F]

            eps_t = const.tile([128, 1], F32, tag="epsc", name="epsc")
            nc.vector.memset(eps_t[:], EPS)
            ones_t = const.tile([32, SPAN], F16, tag="ones", name="ones")
            nc.vector.memset(ones_t[:], 1.0)
            # PE p-state warm-up: keep the PE busy during the initial x-slab
            # load so span-0 matmuls run at full clock.
            for _ in range(WARMUP):
                wm = ops1.tile([128, 512], F32, tag="op1", name="wm")
                nc.tensor.matmul(
                    wm[0:128, 0:192], ones_t[0:32, 0:128],
                    ones_t[0:32, 0:192], start=True, stop=True)
            if DUMMY_SQRT:
                # dummy Sqrt: makes the compiler pick the sqrt-bearing
                # activation table up front (it also holds Square/Copy)
                scr0 = const.tile([1, 1], F32, tag="scr0", name="scr0")
                nc.scalar.activation(scr0[0:1, 0:1], eps_t[0:1, 0:1],
                                     Act.Sqrt, bias=eps_t[0:1, 0:1],
                                     scale=1.0)

            # persistent double-buffered lhsT tiles; the ones row (row F)
            # is written once via 32-aligned copies (clobbered x rows are
            # rewritten by the per-span q prep before use).
            xpt = {}
            for i, p in enumerate(PACKS):
                for par in range(2):
                    t_ = const.tile([p.K32, SPAN], F16, tag=f"xp{p.pid}_{par}",
                                    name=f"xp{p.pid}_{par}")
                    xpt[(p.pid, par)] = t_
                    for m0 in range(p.F // 32 * 32, p.K32, 32):
                        if (i + par) % 2 == 0:
                            nc.vector.tensor_copy(t_[m0:m0 + 32, :],
                                                  ones_t[0:32, :])
                        else:
                            nc.scalar.activation(t_[m0:m0 + 32, :],
                                                 ones_t[0:32, :], Act.Copy)

            from concourse.dve_ops import (
                RECIP_APPROX_FAST_CONSTS as _RC,
                RECIPROCAL_APPROX_FAST as _RF,
            )
            srstd_by_si = {}

            def emit_stats(si, qs):
                """Stats + rstd chain for span si, ONE qset (3 packs).  The
                two qsets serialize through single-buffered [96, SPAN] PSUM
                tiles (2 banks total), and are emitted in different chunk
                slots so the PE never stalls on the Act/DVE tail."""
                s0, sw = SPANS[si]
                srstd16 = srp.tile([96, SPAN], F16, tag=f"sr16{qs}",
                                   name=f"sr16{qs}")
                srstd_by_si.setdefault(si, {})[qs] = srstd16
                mu_ps = stps.tile([96, SPAN], F32, tag="mu", name="mu")
                msq_ps = stps.tile([96, SPAN], F32, tag="ms", name="ms")
                for p in PACKS:
                    if p.qset != qs:
                        continue
                    q0 = 32 * p.quad
                    xin_f = xin[p.pid][:, s0:s0 + sw]
                    # x^2 on GPSIMD (SBUF->SBUF is legal there), freeing the
                    # scalar engine for PSUM drains
                    xsq = xsqp.tile([128, SPAN], F16, tag="xsq", name="xsq")
                    nc.gpsimd.tensor_tensor(
                        xsq[0:p.F, :sw], xin_f, xin_f, op=Alu.mult)
                    nc.tensor.matmul(
                        mu_ps[q0:q0 + 32, :sw], selx(p),
                        xin_f, start=True, stop=True)
                    nc.tensor.matmul(
                        msq_ps[q0:q0 + 32, :sw], selx(p),
                        xsq[0:p.F, :sw], start=True, stop=True)
                # rstd = rsqrt(msq - mu^2 + eps), batched over the qset
                musq = stt.tile([96, SPAN], F32, tag=f"musq{qs}",
                                name=f"musq{qs}")
                nc.scalar.activation(
                    musq[0:96, :sw], mu_ps[0:96, :sw], Act.Square)
                var = stt.tile([96, SPAN], F32, tag="var", name="var")
                nc.vector.tensor_tensor(
                    var[0:96, :sw], msq_ps[0:96, :sw],
                    musq[0:96, :sw], op=Alu.subtract)
                sq = stt.tile([96, SPAN], F32, tag="sq", name="sq")
                nc.scalar.activation(
                    sq[0:96, :sw], var[0:96, :sw], Act.Sqrt,
                    bias=eps_t[0:96, 0:1], scale=1.0)
                srstd = stt.tile([96, SPAN], F32, tag="sr", name="sr")
                nc.vector._custom_dve(
                    _RF, out=srstd[0:96, :sw],
                    in0=sq[0:96, :sw], s0=_RC["s0"], s1=_RC["s1"],
                    imm2=_RC["imm2"])
                # fp16 copy for the (all-fp16) erep broadcast matmul;
                # on GPSIMD to keep Act/DVE free for drains
                nc.gpsimd.tensor_copy(srstd16[0:96, :sw],
                                      srstd[0:96, :sw])

            def emit_prep(si, qs):
                """Pack lhsT prep for span si, one qset: q = x * rstd_rep,
                into the parity-si xpt tiles.  Emitted during span si-1 (the
                tiles are double-buffered) so span si's mains start
                immediately; split by qset so the last chunk's drains only
                wait on the B half."""
                s0, sw = SPANS[si]
                par = si % 2
                srstd16 = srstd_by_si[si].pop(qs)
                for p in PACKS:
                    if p.qset != qs:
                        continue
                    q0 = 32 * p.quad
                    t_ = xpt[(p.pid, par)]
                    rr = ops1.tile([128, 512], F32, tag="op1", name="rr")
                    nc.tensor.matmul(
                        rr[0:p.F, 0:sw],
                        erep(p),
                        srstd16[q0:q0 + p.n, :sw],
                        start=True, stop=True)
                    nc.vector.tensor_tensor(
                        t_[0:p.F, :sw], xin[p.pid][:, s0:s0 + sw],
                        rr[0:p.F, 0:sw], op=Alu.mult)

            emit_stats(0, 'A')
            emit_stats(0, 'B')
            emit_prep(0, 'A')
            emit_prep(0, 'B')
            for si, (s0, sw) in enumerate(SPANS):
                par = si % 2
                # ---- main matmuls + merged drains + one DMA per t-chunk;
                # next span's slab/stats/prep interleave between chunks so
                # the in-order engine queues never batch them behind a whole
                # span of drains.
                for ci, c0 in enumerate(range(s0, s0 + sw, CHUNK)):
                    cw = min(CHUNK, s0 + sw - c0)
                    ob = obp.tile([128, OB_COLS], F16, tag="ob", name="ob")
                    for kind, members, nb, w, obcol, eng in DRAIN_GROUPS:
                        pool = ops2 if kind == "op2" else ops1
                        op = pool.tile([128, 512 * (2 if kind == "op2" else 1)],
                                       F32, tag=kind, name="op")
                        for pid, blocks in members:
                            p = PACKS[pid]
                            lhsT = xpt[(p.pid, par)][0:p.K,
                                                     c0 - s0:c0 - s0 + cw]
                            for (bank, poff, bw_) in blocks:
                                wcol = p.k0 * C + poff
                                nc.tensor.matmul(
                                    op[0:cw, 512 * bank:512 * bank + bw_],
                                    lhsT,
                                    wext[0:p.K, wcol:wcol + bw_],
                                    start=True, stop=True)
                        src = op[0:cw, 0:512 * nb].rearrange(
                            "t (b x) -> t b x", b=nb)[:, :, 0:w]
                        dst = ob[0:cw, obcol:obcol + nb * w].rearrange(
                            "t (b x) -> t b x", b=nb)
                        if eng == "scalar":
                            nc.scalar.activation(dst, src, Act.Copy)
                        else:
                            nc.vector.tensor_copy(dst, src)
                    nc.sync.dma_start(
                        out=out_d[c0:c0 + cw, :, :].rearrange(
                            "t j c -> t (j c)"),
                        in_=ob[0:cw, :])
                    if ci == 0:
                        if _s0w <= si + 2 < len(SPANS):
                            load_slab(*SPANS[si + 2])
                        if si + 1 < len(SPANS):
                            emit_stats(si + 1, 'A')
                    if ci == 1 and si + 1 < len(SPANS):
                        emit_stats(si + 1, 'B')
                    if ci == 2 and si + 1 < len(SPANS):
                        emit_prep(si + 1, 'A')
                if si + 1 < len(SPANS):
                    emit_prep(si + 1, 'B')

    nc.compile()
    _CACHE["nc"] = nc
    return nc


# ------------------------------------------------------------------ driver --
def kernel(**inputs):
    from concourse.bass_utils import run_bass_kernel_spmd

    x = np.asarray(inputs["x"]).astype(np.float16)
    B = x.shape[0]
    assert x.shape == (8, 2, F_BINS, T)
    ext = _host_params(inputs)
    nc = _build()
    in_maps = []
    for b in range(B):
        m = {"x": x[b]}
        m.update(ext)
        in_maps.append(m)
    res = run_bass_kernel_spmd(nc, in_maps, core_ids=list(range(8)))
    out = np.stack([res.results[b]["out"].transpose(1, 0, 2)
                    for b in range(B)], axis=0)
    return out.astype(np.float32)
